# revision 39
# baseline (speedup 1.0000x reference)
"""Blended-expert MLP (MoE routing) Trainium2 Bass kernel.

Math: reference computes, per layer,
    h = elu( einsum("bi,bio->bo", x, einsum("be,eio->bio", c, w)) + c @ b )
which factorizes as
    h = elu( sum_e (c[:,e] * x) @ W_e  +  c @ b )
(row-scaling commutes with the matmul), so per layer we scale X^T by
c_e on the vector engine and run 8 [rows,512]x[512,512] matmuls plus
one tiny K=8 matmul for the blended bias, ALL accumulating into a
single PSUM tile. Then ELU, then a PE transpose to produce the next
layer's stationary operand.

Sharding: data-parallel over the batch. B=512 rows split across 8
NeuronCores (64 rows each); the expert weights are replicated to every
core. No collectives (on-chip AllReduce has a ~20us latency floor).

int8 weight path (the big lever vs the fp16 baseline): the weight DMA
is the wall (fp16 = 12.6 MB/core at ~358 GB/s/core = 35 us). Weights
are uniform-distributed, so int8 quantization costs only ~0.4%%/layer
(measured 6.8e-3 total rel err vs the 2e-2 budget) and halves the DMA
to 6.3 MB (~18.6 us). The PE has no int8 mode, so weights are widened
on-chip to EXACT fp16 integers in [-127,127]:
  - DVE chunks ride as uint16 byte-pairs (host packs col j of experts
    0-3 into the low byte, col j of experts 4-7 into the high byte);
    two tensor_scalar ops split them: (v & 0xFF) - 128 and
    (v >> 8) - 128. All operands 2-byte -> DVE 4x mode, ~1.2us/chunk.
  - ACT / GPSIMD chunks ride as plain uint8; one activation
    Copy(in - 128) / tensor_scalar subtract per half, ~3.5us/chunk.
  Assignment per layer: k=0,3 -> DVE (layer closers need low latency),
  k=1 -> GPSIMD, k=2 -> ACT.
The float scale s_w (per layer, max|w|/127) is applied at PSUM
evacuation time via a per-partition fp32 scale AP (runtime data, so
the NEFF needs no recompile if inputs change); biases are pre-divided
by s_w on the host (they are structurally zero in this problem).

fp16 x/matmul path unchanged from the baseline: ~7e-4 of the error
budget. PE warmup matmuls get the HAM clock gate to 2.4 GHz before
real work; k-outer matmul order fires each chunk's matmuls on widen
completion; even/odd experts run concurrently in the two column
halves of the PE array.
"""

import numpy as np

B, E, D = 512, 8, 512
NCORES = 8
ROWS = B // NCORES  # 64
KC = D // 128  # 4 contraction chunks of 128
NCHUNK = 3 * KC  # 12

# pack tensor column layout (per 128 partitions)
PK_XT = 0  # [128, 256]: layer-1 x^T chunk k at cols [64k, 64k+64)
PK_CB = 256  # [128, 2048]: c broadcast; col 256e+64j+b = C[b,e], all partitions
PK_ID = PK_CB + E * KC * ROWS  # [64, 64]: identity, partitions 0..63
PK_CT = PK_ID + ROWS  # [8, 64]: coef^T, partitions 0..7
PCK = PK_CT + ROWS

MODE = "i8"
N_WARMUP = 7

# every chunk rides as uint16 byte-pairs: lo byte = experts 0-3, hi byte =
# experts 4-7. DVE extracts the hi half via v/256-128 (693ns, 4x mode); the
# lo half is a stride-2 uint8 subtract: DVE 1.23us / ACT 2.0us measured.
# GPSIMD is catastrophic on integer ops (29us!) but fine on fp16/fp32 SBUF
# ops, so it takes the xs scaling and part of the elu chain instead.
LO_DVE = {0, 1, 4, 5, 8}  # chunks whose lo extract runs on DVE (rest ACT)
# matmul issue order: hi-half experts first (their extract lands first)
ORD_E = [4, 5, 6, 7, 0, 1, 2, 3]
# layer-0 scale ops: first-consumed experts on the fast DVE
DVE_SCALE_E = (4, 5, 6, 7)

_NC_CACHE = {}


def _build(mode):
    from contextlib import ExitStack

    import concourse.bacc as bacc
    import concourse.mybir as mybir
    import concourse.tile as tile

    f32 = mybir.dt.float32
    f16 = mybir.dt.float16
    u16 = mybir.dt.uint16
    u8 = mybir.dt.uint8
    Alu = mybir.AluOpType
    Act = mybir.ActivationFunctionType

    nc = bacc.Bacc()
    pack_d = nc.declare_dram_parameter("pack", [128, PCK], f16, isOutput=False)
    scl_d = nc.declare_dram_parameter("scl", [128, 4], f32, isOutput=False)
    bias_d = nc.declare_dram_parameter("biasd", [E, 3 * D], f16, isOutput=False)
    wpair_d = nc.declare_dram_parameter(
        "wpair", [NCHUNK, 128, E * D // 2], u16, isOutput=False
    )
    out_d = nc.declare_dram_parameter("out", [ROWS, D], f32, isOutput=True)

    HED = E * D // 2  # 2048: wfp column offset of experts 4-7

    with ExitStack() as ctx:
        tc = ctx.enter_context(tile.TileContext(nc))
        const = ctx.enter_context(tc.tile_pool(name="const", bufs=1))
        # all 12 pair chunks live simultaneously: no WAR gating of the DMA
        # stream behind widen consumption (bufs=4 stretched the DMA window
        # from 18.6us to 32us)
        prpool = ctx.enter_context(tc.tile_pool(name="prp", bufs=NCHUNK))
        wfpool = ctx.enter_context(tc.tile_pool(name="wfp", bufs=6))
        spool = ctx.enter_context(tc.tile_pool(name="sp", bufs=18))
        s0pool = ctx.enter_context(tc.tile_pool(name="s0", bufs=1))
        hpool = ctx.enter_context(tc.tile_pool(name="hp", bufs=2))
        xpool = ctx.enter_context(tc.tile_pool(name="xp", bufs=2))
        acc_ps = ctx.enter_context(tc.tile_pool(name="acc", bufs=3, space="PSUM"))
        pt_ps = ctx.enter_context(tc.tile_pool(name="pt", bufs=3, space="PSUM"))
        wm_ps = ctx.enter_context(tc.tile_pool(name="wm", bufs=1, space="PSUM"))

        # PE warmup: garbage matmuls on a zeroed tile (output never read) so
        # the HAM clock gate reaches 2.4 GHz before the first real matmul.
        # memset on DVE (ready ~3.3us) rather than gpsimd (ready later).
        warm = const.tile([128, ROWS + D], f16)
        nc.vector.memset(warm[:], 0.0)
        wps = wm_ps.tile([ROWS, D], f32, tag="warm")
        for _ in range(N_WARMUP):
            nc.tensor.matmul(
                wps[:], warm[:, 0:ROWS], warm[:, ROWS:], start=True, stop=True
            )

        scl_t = const.tile([128, 4], f32)
        nc.sync.dma_start(scl_t[:], scl_d[:])
        pack_t = const.tile([128, PCK], f16)
        nc.sync.dma_start(pack_t[:], pack_d[:])
        bias_t = const.tile([E, 3 * D], f16)
        nc.gpsimd.dma_start(bias_t[:], bias_d[:])

        coeft_ap = pack_t[0:E, PK_CT : PK_CT + ROWS]
        ident_ap = pack_t[0:ROWS, PK_ID : PK_ID + ROWS]
        xt_tile, xt_off = pack_t, PK_XT  # current x^T source: [128, 256] at offset

        # all weight-chunk DMAs up-front in consumption order, spread over
        # three queues: a single queue needs ~10us just to push the 15
        # descriptors (~0.6us each), which starves the DMA engines
        DMA_Q = {3: nc.scalar, 4: nc.scalar, 5: nc.scalar,
                 6: nc.gpsimd, 7: nc.gpsimd, 8: nc.gpsimd}
        raw_tiles = []
        for c in range(NCHUNK):
            q = DMA_Q.get(c, nc.sync)
            wt = prpool.tile([128, HED], u16, tag="wp")
            if c == NCHUNK - 1:
                # split the final chunk so experts 0-1/4-5 land earlier and
                # only the tail experts gate on the very last transfer
                q.dma_start(wt[:, 0 : HED // 2], wpair_d[c, :, 0 : HED // 2])
                q.dma_start(wt[:, HED // 2 :], wpair_d[c, :, HED // 2 :])
            else:
                q.dma_start(wt[:], wpair_d[c, :, :])
            raw_tiles.append(wt)

        def widen(c):
            """Emit widen ops for chunk c; returns the fp16 weight tile."""
            wt = raw_tiles[c]
            wf = wfpool.tile([128, E * D], f16, tag="wf")
            # hi extract on DVE, all-arith (the ISA has no DVE mod/bitwise
            # mixing): v/256 - 128 = w_q_hi + lo_byte/256; the host pre-
            # compensates that leakage into the hi byte.
            nsplit = 2 if c == NCHUNK - 1 else 1
            step = HED // nsplit
            for s in range(nsplit):
                lo, hi = s * step, (s + 1) * step
                nc.vector.tensor_scalar(
                    wf[:, HED + lo : HED + hi], wt[:, lo:hi],
                    1.0 / 256.0, 128.0, Alu.mult, Alu.subtract,
                )
            # lo extract: stride-2 uint8 view minus 128, on ACT or DVE
            lov = wt[:].bitcast(u8).rearrange("p (n two) -> p two n", two=2)
            for s in range(nsplit):
                lo, hi = s * step, (s + 1) * step
                if c in LO_DVE:
                    nc.vector.tensor_scalar(
                        wf[:, lo:hi], lov[:, 0, lo:hi], 128.0, None,
                        Alu.subtract,
                    )
                else:
                    nc.scalar.activation(
                        wf[:, lo:hi], lov[:, 0, lo:hi], Act.Copy, bias=-128.0
                    )
            return wf

        for layer in range(3):
            # widen this layer's chunks (emitted per layer so queued engine
            # work stays roughly in execution order)
            wfs = [widen(layer * KC + k) for k in range(KC)]
            sw_ap = scl_t[0:ROWS, layer : layer + 1]

            # scale x^T by c_e along the batch (free) dim, one full-width op
            # per expert, emitted in consumption order. Layer 0 scales only
            # need pack, so the otherwise-idle GPSIMD does them all; later
            # layers keep the first-consumed experts on the faster DVE.
            # xs = x^T * c_e, written into one [128, E*256] tile indexed
            # e*256 + 64k + b. Layer 0 has xt in full (pack): per-expert
            # full-width ops, first-consumed experts on DVE, rest GPSIMD.
            # Layers 1-2 scales are emitted per quarter inside the previous
            # boundary (see below) so matmuls restart early.
            SCW = KC * ROWS  # 256
            if layer == 0:
                sc_all = s0pool.tile([128, E * SCW], f16, tag="sc0")
                xt_ap = xt_tile[:, xt_off : xt_off + SCW]
                for e in ORD_E:
                    eng = nc.vector if e in DVE_SCALE_E else nc.gpsimd
                    eng.tensor_tensor(
                        out=sc_all[:, SCW * e : SCW * (e + 1)],
                        in0=xt_ap,
                        in1=pack_t[:, PK_CB + SCW * e : PK_CB + SCW * (e + 1)],
                        op=Alu.mult,
                    )
                scaled = [(sc_all, e * SCW) for e in range(E)]
            # else: `scaled` was produced by the previous layer's boundary

            # one accumulation group: 32 expert matmuls + bias matmul (K=8).
            # k-outer order: each chunk's 8 expert matmuls fire as soon as
            # its widen lands. Even/odd experts run CONCURRENTLY in the two
            # column halves of the PE array (tile_position); the partition
            # halves of acc are summed afterwards.
            acc = acc_ps.tile([2 * ROWS, D], f32, tag="acc")
            nc.tensor.matmul(
                acc[0:ROWS, :],
                coeft_ap,
                bias_t[:, D * layer : D * (layer + 1)],
                start=True,
                stop=False,
                tile_position=(0, 0),
                skip_group_check=True,
            )
            for k in range(KC):
                for e in ORD_E:
                    half = e % 2
                    sct, sco = scaled[e]
                    nc.tensor.matmul(
                        acc[half * ROWS : (half + 1) * ROWS, :],
                        sct[:, sco + ROWS * k : sco + ROWS * (k + 1)],
                        wfs[k][:, D * e : D * (e + 1)],
                        start=(k == 0 and e == ORD_E[1]),
                        stop=(k == KC - 1 and e in ORD_E[-2:]),
                        tile_position=(0, half * ROWS),
                        skip_group_check=True,
                    )

            # evacuate even half with the s_w scale (ACT), merge+scale the
            # odd half (DVE stt), elu, transpose; pipelined per 128-column
            # quarter so each quarter flows through the chain independently
            t0 = hpool.tile([ROWS, D], f32, tag="t0")
            hpre = hpool.tile([ROWS, D], f32, tag="hpre")
            HD = D // 2
            if layer < 2:
                ex = hpool.tile([ROWS, D], f32, tag="ex")
                h = hpool.tile([ROWS, D], f16, tag="h")
                xt_t = xpool.tile([128, KC * ROWS], f16, tag="xt")
                # full-width merge/elu chain (per-op overhead dominates
                # smaller slices)
                nc.scalar.activation(
                    t0[:], acc[0:ROWS, :], Act.Copy, scale=sw_ap
                )
                nc.vector.scalar_tensor_tensor(
                    out=hpre[:],
                    in0=acc[ROWS:, :],
                    scalar=sw_ap,
                    in1=t0[:],
                    op0=Alu.mult,
                    op1=Alu.add,
                )
                nc.scalar.activation(ex[:], hpre[:], Act.Exp)
                nc.vector.tensor_scalar(
                    ex[:], ex[:], 1.0, 0.0, Alu.subtract, Alu.min
                )
                nc.vector.scalar_tensor_tensor(
                    out=h[:],
                    in0=hpre[:],
                    scalar=0.0,
                    in1=ex[:],
                    op0=Alu.max,
                    op1=Alu.add,
                )
                # transpose -> evacuate -> scale ALL experts, per quarter:
                # the next layer's chunk-k matmuls need only quarter k, so
                # the PE restarts ~3us into the boundary instead of ~10
                sc_next = s0pool.tile([128, E * SCW], f16, tag="sc0")
                for q in range(KC):
                    qs = slice(128 * q, 128 * (q + 1))
                    pt = pt_ps.tile([128, ROWS], f16, tag="pt")
                    nc.tensor.transpose(pt[:], h[:, qs], ident_ap)
                    dst = xt_t[:, ROWS * q : ROWS * (q + 1)]
                    if q % 2 == 0:
                        nc.scalar.copy(dst, pt[:])
                    else:
                        nc.vector.tensor_copy(dst, pt[:])
                    # one 3D op scales quarter q for all 8 experts
                    nc.gpsimd.tensor_tensor(
                        out=sc_next[:]
                        .rearrange("p (e c) -> p e c", e=E)[
                            :, :, ROWS * q : ROWS * (q + 1)
                        ],
                        in0=dst.unsqueeze(1).broadcast_to((128, E, ROWS)),
                        in1=pack_t[:, PK_CB : PK_CB + E * SCW]
                        .rearrange("p (e c) -> p e c", e=E)[
                            :, :, ROWS * q : ROWS * (q + 1)
                        ],
                        op=Alu.mult,
                    )
                xt_tile, xt_off = xt_t, 0
                scaled = [(sc_next, e * SCW) for e in range(E)]
            else:
                # stream the output per column half, right behind the merge
                for cc in range(2):
                    cs = slice(HD * cc, HD * (cc + 1))
                    nc.scalar.activation(
                        t0[:, cs], acc[0:ROWS, cs], Act.Copy, scale=sw_ap
                    )
                    nc.vector.scalar_tensor_tensor(
                        out=hpre[:, cs],
                        in0=acc[ROWS:, cs],
                        scalar=sw_ap,
                        in1=t0[:, cs],
                        op0=Alu.mult,
                        op1=Alu.add,
                    )
                    nc.sync.dma_start(out_d[:, cs], hpre[:, cs])

    nc.compile()
    return nc


def _get_nc(mode=MODE):
    if mode not in _NC_CACHE:
        _NC_CACHE[mode] = _build(mode)
    return _NC_CACHE[mode]


def _prep_in_maps(inputs, mode=MODE):
    X = np.asarray(inputs["X"], np.float32)
    C = np.asarray(inputs["blending_coef"], np.float32)
    ws = [np.asarray(inputs[f"w_l{i}"], np.float32) for i in (1, 2, 3)]
    bs = [np.asarray(inputs[f"b_l{i}"], np.float32) for i in (1, 2, 3)]

    # W[l][i, e*D+o] = w_l[e, i, o]; int8-quantize per layer
    sw = np.array([max(np.abs(w).max() / 127.0, 1e-30) for w in ws], np.float32)
    scaled_ws = []
    for l, w in enumerate(ws):
        W = w.transpose(1, 0, 2).reshape(D, E * D)
        scaled_ws.append((W / sw[l]).astype(np.float64))  # in [-127, 127]
    wpair = np.zeros((NCHUNK, 128, E * D // 2), np.uint16)
    HED = E * D // 2
    for c in range(NCHUNK):
        l, k = c // KC, c % KC
        sub = scaled_ws[l][128 * k : 128 * (k + 1)]
        # pair layout: lo byte = experts 0-3, hi byte = experts 4-7. The
        # on-chip hi extract is v/256 - 128 = w_hi + lo/256, so pre-
        # subtract the known lo/256 leakage before rounding.
        a = (np.round(sub[:, :HED]).clip(-127, 127) + 128.0).astype(np.uint16)
        b = np.round(sub[:, HED:] + 128.0 - a / 256.0).clip(0, 255)
        wpair[c] = a | (b.astype(np.uint16) << 8)

    Bb = np.concatenate([b / s for b, s in zip(bs, sw)], axis=1).astype(
        np.float16
    )  # [E, 3*D], pre-divided by s_w (zeros in this problem)
    scl = np.broadcast_to(
        np.concatenate([sw, [1.0]]).astype(np.float32), (128, 4)
    ).copy()

    in_maps = []
    for c in range(NCORES):
        rs = slice(c * ROWS, (c + 1) * ROWS)
        pack = np.zeros((128, PCK), np.float32)
        # xt chunks: pack[p, 64k+b] = X[rows][b, 128k+p]
        xt = np.ascontiguousarray(X[rs].T)  # [512, 64]
        pack[:, PK_XT : PK_XT + KC * ROWS] = (
            xt.reshape(KC, 128, ROWS).transpose(1, 0, 2).reshape(128, KC * ROWS)
        )
        # c broadcast: pack[p, PK_CB + 256e + 64j + b] = C[rs][b, e]
        pack[:, PK_CB : PK_CB + E * KC * ROWS] = np.broadcast_to(
            C[rs].T[:, None, :], (E, KC, ROWS)
        ).reshape(1, E * KC * ROWS)
        pack[0:ROWS, PK_ID : PK_ID + ROWS] = np.eye(ROWS, dtype=np.float32)
        pack[0:E, PK_CT : PK_CT + ROWS] = C[rs].T
        in_maps.append(
            {
                "pack": pack.astype(np.float16),
                "biasd": Bb,
                "wpair": wpair,
                "scl": scl,
            }
        )
    return in_maps


def run(inputs, mode=MODE, trace=False):
    """Returns (output [512,512] fp32, BassKernelResults)."""
    from concourse.bass_utils import run_bass_kernel_spmd

    nc = _get_nc(mode)
    in_maps = _prep_in_maps(inputs, mode)
    res = run_bass_kernel_spmd(nc, in_maps, list(range(NCORES)), trace=trace)
    out = np.concatenate([r["out"] for r in res.results], axis=0)
    return out, res


def kernel(**inputs) -> np.ndarray:
    out, _ = run(inputs)
    return out


# revision 47
# speedup vs baseline: 1.1332x; 1.1332x over previous
"""Blended-expert MLP (MoE routing) Trainium2 Bass kernel.

Math: reference computes, per layer,
    h = elu( einsum("bi,bio->bo", x, einsum("be,eio->bio", c, w)) + c @ b )
which factorizes as
    h = elu( sum_e (c[:,e] * x) @ W_e  +  c @ b )
(row-scaling commutes with the matmul), so per layer we scale X^T by
c_e on the vector engine (8 ops) and run 8 [rows,512]x[512,512]
matmuls plus one tiny K=8 matmul for the blended bias, ALL accumulating
into a single PSUM tile. Then ELU, then a PE transpose to produce the
next layer's stationary operand.

Sharding: data-parallel over the batch. B=512 rows split across 8
NeuronCores (64 rows each); the expert weights are replicated to every
core (fp16: 12 MB/core, fully SBUF-resident). No collectives (on-chip
AllReduce has a ~20us latency floor, worse than replication).

Layout per core:
  stationary operand = (c_e * X)^T chunks [128(i), 64(b)]
  moving operand     = W chunks  [128(i), 4096(e,o)] sliced per expert
  psum out           = [128(2 expert-halves x 64b), 512(o)], fp32

fp16 operands (not bf16): same DMA bytes and same 1-cycle/row matmul
rate, but 10 mantissa bits give ~7e-4 relative error vs ~6e-3.

Performance structure (measured ~50-54us on hardware, best 49.6us):
  ~7.5us fixed NEFF preamble | ~33.5us weight-DMA window (the wall:
  12 MB at ~360 GB/s/core, fully overlapped with compute) | ~4us
  compute tail | ~6us fixed all-engine end barrier.
Tricks that matter: PE warmup matmuls (HAM clock gate: cold PE runs at
1.2 GHz, warm 2.4 GHz), k-outer matmul order (each weight chunk's
matmuls fire on DMA arrival), even/odd expert pairs running
concurrently in the two column halves of the PE array (M=64 would
otherwise idle half the array), and a column-halved pipeline for the
psum-merge + ELU boundary chain.
"""

import numpy as np

B, E, D = 512, 8, 512
NCORES = 8
ROWS = B // NCORES  # 64
KC = D // 128  # 4 contraction chunks of 128

# pack tensor column layout (per 128 partitions)
PK_XT = 0  # [128, 256]: layer-1 x^T chunk k at cols [64k, 64k+64)
# c broadcast, one block per layer; col 1024l+128e+64j+b = C[b,e] times the
# layer's weight-dequant factor (s_w*1024 for int8 layers 0-1, 1 for fp16
# layer 2) - folding the scale here keeps the PSUM/evacuation path identical
PK_CB = 256
PK_ID = PK_CB + 3 * E * 2 * ROWS  # [64, 64]: identity, partitions 0..63
PK_CT = PK_ID + ROWS  # [8, 64]: coef^T, partitions 0..7
PCK = PK_CT + ROWS

MODE = "f16"
N8_LAYERS = 2  # layers 0..N8_LAYERS-1 ship weights as int8 byte-pairs
LO_DVE = {0, 1}  # pair chunks whose lo-byte extract runs on DVE (rest ACT)

_NC_CACHE = {}


def _mmdt(mybir, mode):
    return {
        "f32": mybir.dt.float32,
        "f32r": mybir.dt.float32r,
        "bf16": mybir.dt.bfloat16,
        "f16": mybir.dt.float16,
    }[mode]


def _build(mode):
    from contextlib import ExitStack

    import concourse.bacc as bacc
    import concourse.mybir as mybir
    import concourse.tile as tile

    f32 = mybir.dt.float32
    mmdt = _mmdt(mybir, mode)
    Alu = mybir.AluOpType
    Act = mybir.ActivationFunctionType

    # Bacc (not raw Bass): its compile() legalizes the TRN2 one-sync-wait-
    # per-instruction limit by splitting excess waits into EventSemaphores
    nc = bacc.Bacc()
    u16 = mybir.dt.uint16
    u8 = mybir.dt.uint8
    N8C = N8_LAYERS * KC  # int8 pair chunks (layers 0-1)
    pack_d = nc.declare_dram_parameter("pack", [128, PCK], mmdt, isOutput=False)
    bias_d = nc.declare_dram_parameter("biasd", [E, 3 * D], mmdt, isOutput=False)
    # layers 0-1: uint16 byte pairs (lo byte = experts 0-3, hi = 4-7),
    # widened on-chip to w_q/1024 in fp16; layer 2: direct fp16
    wpair_d = nc.declare_dram_parameter(
        "wpair", [N8C, 128, E * D // 2], u16, isOutput=False
    )
    w_d = nc.declare_dram_parameter(
        "w", [3 - N8_LAYERS, D, E * D], mmdt, isOutput=False
    )
    out_d = nc.declare_dram_parameter("out", [ROWS, D], f32, isOutput=True)
    HED = E * D // 2

    with ExitStack() as ctx:
        tc = ctx.enter_context(tile.TileContext(nc))
        const = ctx.enter_context(tc.tile_pool(name="const", bufs=1))
        wpool = ctx.enter_context(tc.tile_pool(name="wp", bufs=4))
        prpool = ctx.enter_context(tc.tile_pool(name="prp", bufs=8))
        wfpool = ctx.enter_context(tc.tile_pool(name="wfp", bufs=6))
        spool = ctx.enter_context(tc.tile_pool(name="sp", bufs=24))
        hpool = ctx.enter_context(tc.tile_pool(name="hp", bufs=2))
        xpool = ctx.enter_context(tc.tile_pool(name="xp", bufs=2))
        acc_ps = ctx.enter_context(tc.tile_pool(name="acc", bufs=3, space="PSUM"))
        pt_ps = ctx.enter_context(tc.tile_pool(name="pt", bufs=3, space="PSUM"))
        wm_ps = ctx.enter_context(tc.tile_pool(name="wm", bufs=1, space="PSUM"))

        # PE warmup: garbage matmuls on a zeroed tile (output never read),
        # emitted first so the HAM clock gate reaches 2.4 GHz before the
        # first real matmul (cold PE at 1.2 GHz otherwise doubles every
        # matmul). gpsimd is free right after its ~3.3us start preamble, so
        # it provides the earliest possible writer for the warm tile.
        warm = const.tile([128, ROWS + D], mmdt)
        nc.gpsimd.memset(warm[:], 0.0)
        wps = wm_ps.tile([ROWS, D], f32, tag="warm")
        for _ in range(14):
            nc.tensor.matmul(
                wps[:], warm[:, 0:ROWS], warm[:, ROWS:], start=True, stop=True
            )

        pack_t = const.tile([128, PCK], mmdt)
        pack_dma = nc.sync.dma_start(pack_t[:], pack_d[:])
        bias_t = const.tile([E, 3 * D], mmdt)
        nc.gpsimd.dma_start(bias_t[:], bias_d[:])

        coeft_ap = pack_t[0:E, PK_CT : PK_CT + ROWS]
        ident_ap = pack_t[0:ROWS, PK_ID : PK_ID + ROWS]
        xt_tile, xt_off = pack_t, PK_XT  # current x^T source: [128, 256] at offset

        # all weight-chunk DMAs up-front; the HWDGE lane round-robin plus
        # issue order paces them in consumption order at full bandwidth
        # (explicit chaining adds ~2us completion-latency per hop - worse)
        raw_pairs = []
        all_wts = []
        for layer in range(3):
            for k in range(KC):
                c = layer * KC + k
                rs_ = slice(128 * k, 128 * (k + 1))
                if layer < N8_LAYERS:
                    pr = prpool.tile([128, HED], u16, tag="wp")
                    nc.sync.dma_start(pr[:], wpair_d[c, :, :])
                    raw_pairs.append(pr)
                    all_wts.append(None)  # widened per layer below
                else:
                    wt = wpool.tile([128, E * D], mmdt, tag="w")
                    l2 = layer - N8_LAYERS
                    if layer == 2 and k == KC - 1:
                        # split the final chunk: its first half (experts
                        # 0-3) lands earlier, so only experts 4-7's last
                        # matmuls gate on the very last transfer
                        nc.sync.dma_start(
                            wt[:, 0:HED], w_d[l2, rs_, 0:HED]
                        )
                        nc.sync.dma_start(wt[:, HED:], w_d[l2, rs_, HED:])
                    else:
                        nc.sync.dma_start(wt[:], w_d[l2, rs_, :])
                    all_wts.append(wt)

        def widen(c):
            """Widen pair chunk c to fp16 w_q/1024 (exact in fp16)."""
            pr = raw_pairs[c]
            wf = wfpool.tile([128, E * D], mmdt, tag="wf")
            # hi: v/(256*1024) - 0.125 = (w_q + lo/256)/1024, lo-leakage
            # pre-compensated on the host
            nc.vector.tensor_scalar(
                wf[:, HED:], pr[:], 1.0 / (256.0 * 1024.0), 0.125,
                Alu.mult, Alu.subtract,
            )
            # lo: u8/1024 - 0.125 from the stride-2 byte view
            lov = pr[:].bitcast(u8).rearrange("p (n two) -> p two n", two=2)
            if c in LO_DVE:
                nc.vector.tensor_scalar(
                    wf[:, 0:HED], lov[:, 0, :], 1.0 / 1024.0, 0.125,
                    Alu.mult, Alu.subtract,
                )
            else:
                nc.scalar.activation(
                    wf[:, 0:HED], lov[:, 0, :], Act.Copy,
                    scale=1.0 / 1024.0, bias=-0.125,
                )
            return wf

        for layer in range(3):
            if layer < N8_LAYERS:
                wts = [widen(layer * KC + k) for k in range(KC)]
            else:
                wts = all_wts[layer * KC : (layer + 1) * KC]

            # scale x^T by c_e along the batch (free) dim: one DVE op per
            # expert over all 4 chunks at once
            # per-chunk rescale: TT(e,k) gates only on evacuation k of the
            # previous layer's transpose, and matmul (e,k) gates only on
            # TT(e,k) (subtile column tracking), so the boundary pipelines
            # at chunk granularity. c-broadcast is stored once (64 cols/e).
            # per chunk-pair rescale: TT(e,half) gates on the first/last two
            # transpose evacuations only, and matmul (e,k) gates on its half
            # (subtile column tracking)
            scaled = []
            for e in range(E):
                sc = spool.tile([128, KC * ROWS], mmdt, tag="sc")
                for half in range(2):
                    lo, hi = 2 * ROWS * half, 2 * ROWS * (half + 1)
                    cbo = PK_CB + 2 * ROWS * (E * layer + e)
                    nc.vector.tensor_tensor(
                        out=sc[:, lo:hi],
                        in0=xt_tile[:, xt_off + lo : xt_off + hi],
                        in1=pack_t[:, cbo : cbo + 2 * ROWS],
                        op=Alu.mult,
                    )
                scaled.append(sc)

            # one accumulation group: 32 expert matmuls + bias matmul (K=8).
            # k-outer order: each weight chunk's 8 expert matmuls fire as
            # soon as that chunk's DMA lands, overlapping the next transfer.
            # Even/odd experts run CONCURRENTLY in the two column halves of
            # the PE array (tile_position), since M=64 only fills half the
            # array; the partition halves of acc are summed afterwards.
            acc = acc_ps.tile([2 * ROWS, D], f32, tag="acc")
            # bias matmul opens the even-half group so the even half is done
            # (and can evacuate) while the last odd matmuls still run
            nc.tensor.matmul(
                acc[0:ROWS, :],
                coeft_ap,
                bias_t[:, D * layer : D * (layer + 1)],
                start=True,
                stop=False,
                tile_position=(0, 0),
                skip_group_check=True,
            )
            for k in range(KC):
                for e in range(E):
                    half = e % 2
                    nc.tensor.matmul(
                        acc[half * ROWS : (half + 1) * ROWS, :],
                        scaled[e][:, ROWS * k : ROWS * (k + 1)],
                        wts[k][:, D * e : D * (e + 1)],
                        start=(k == 0 and e == 1),
                        stop=(k == KC - 1 and e >= E - 2),
                        tile_position=(0, half * ROWS),
                        skip_group_check=True,
                    )
            # evacuate even half (ACT) + merge halves (DVE) + elu + transpose,
            # pipelined per 128-column quarter: transpose k consumes exactly
            # quarter k, so each quarter flows through the whole boundary
            # chain independently
            t0 = hpool.tile([ROWS, D], f32, tag="t0")
            hpre = hpool.tile([ROWS, D], f32, tag="hpre")
            HD = D // 2
            if layer < 2:
                # keep the PE clock warm across the elu/transpose boundary
                # (a >3.4us PE-idle window would re-throttle to 1.2 GHz)
                for _ in range(8):
                    nc.tensor.matmul(
                        wps[:], warm[:, 0:ROWS], warm[:, ROWS:],
                        start=True, stop=True,
                    )

            if layer < 2:
                # per quarter q: copy+merge, elu(x)=max(x,0)+min(exp(x)-1,0),
                # then transpose + evacuation - all stages pipeline across
                # quarters on alternating engines
                ex = hpool.tile([ROWS, D], f32, tag="ex")
                h = hpool.tile([ROWS, D], mmdt, tag="h")
                xt_t = xpool.tile([128, KC * ROWS], mmdt, tag="xt")
                for q in range(KC):
                    qs = slice(128 * q, 128 * (q + 1))
                    nc.scalar.copy(t0[:, qs], acc[0:ROWS, qs])
                    nc.vector.tensor_tensor(
                        out=hpre[:, qs], in0=t0[:, qs], in1=acc[ROWS:, qs],
                        op=Alu.add,
                    )
                    nc.scalar.activation(ex[:, qs], hpre[:, qs], Act.Exp)
                    nc.vector.tensor_scalar(
                        ex[:, qs], ex[:, qs], 1.0, 0.0, Alu.subtract, Alu.min
                    )
                    nc.vector.scalar_tensor_tensor(
                        out=h[:, qs],
                        in0=hpre[:, qs],
                        scalar=0.0,
                        in1=ex[:, qs],
                        op0=Alu.max,
                        op1=Alu.add,
                    )
                    pt = pt_ps.tile([128, ROWS], mmdt, tag="pt")
                    nc.tensor.transpose(pt[:], h[:, qs], ident_ap)
                    dst = xt_t[:, ROWS * q : ROWS * (q + 1)]
                    if q % 2 == 0:
                        nc.scalar.copy(dst, pt[:])
                    else:
                        nc.vector.tensor_copy(dst, pt[:])
                xt_tile, xt_off = xt_t, 0
            else:
                # stream the output per column half, right behind the merge
                for c in range(2):
                    cs = slice(HD * c, HD * (c + 1))
                    nc.scalar.copy(t0[:, cs], acc[0:ROWS, cs])
                    nc.vector.tensor_tensor(
                        out=hpre[:, cs], in0=t0[:, cs], in1=acc[ROWS:, cs],
                        op=Alu.add,
                    )
                    nc.sync.dma_start(out_d[:, cs], hpre[:, cs])

    nc.compile()
    return nc


def _get_nc(mode):
    if mode not in _NC_CACHE:
        _NC_CACHE[mode] = _build(mode)
    return _NC_CACHE[mode]


def _prep_in_maps(inputs, mode):
    X = np.asarray(inputs["X"], np.float32)
    C = np.asarray(inputs["blending_coef"], np.float32)
    ws = [np.asarray(inputs[f"w_l{i}"], np.float32) for i in (1, 2, 3)]
    bs = [np.asarray(inputs[f"b_l{i}"], np.float32) for i in (1, 2, 3)]
    mm_np = np.float16

    # layers 0..N8_LAYERS-1: int8 byte-pairs; the rest: direct fp16.
    # W[l][i, e*D+o] = w_l[e, i, o]
    HED = E * D // 2
    sw = np.ones(3, np.float32)
    wpair = np.zeros((N8_LAYERS * KC, 128, HED), np.uint16)
    for l in range(N8_LAYERS):
        Wl = ws[l].transpose(1, 0, 2).reshape(D, E * D)
        sw[l] = max(np.abs(Wl).max() / 127.0, 1e-30)
        Wq = (Wl / sw[l]).astype(np.float64)
        for k in range(KC):
            sub = Wq[128 * k : 128 * (k + 1)]
            # lo byte = experts 0-3; hi byte = experts 4-7. On-chip hi
            # extract is v/256 - 128 (in w_q units), leaking lo/256: pre-
            # subtract it before rounding.
            a = (np.round(sub[:, :HED]).clip(-127, 127) + 128.0).astype(
                np.uint16
            )
            b = np.round(sub[:, HED:] + 128.0 - a / 256.0).clip(0, 255)
            wpair[l * KC + k] = a | (b.astype(np.uint16) << 8)
    W = np.stack(
        [
            w.transpose(1, 0, 2).reshape(D, E * D)
            for w in ws[N8_LAYERS:]
        ]
    ).astype(mm_np)
    Bb = np.concatenate(bs, axis=1).astype(mm_np)  # [E, 3*D]

    # per-layer c-broadcast factor: widened weights are w_q/1024, so the
    # xs side carries s_w*1024 (safely inside fp16 normal range)
    cfac = np.where(np.arange(3) < N8_LAYERS, sw * 1024.0, 1.0)

    in_maps = []
    for c in range(NCORES):
        rs = slice(c * ROWS, (c + 1) * ROWS)
        pack = np.zeros((128, PCK), np.float32)
        # xt chunks: pack[p, 64k+b] = X[rows][b, 128k+p]
        xt = np.ascontiguousarray(X[rs].T)  # [512, 64]
        pack[:, PK_XT : PK_XT + KC * ROWS] = (
            xt.reshape(KC, 128, ROWS).transpose(1, 0, 2).reshape(128, KC * ROWS)
        )
        # c broadcast per layer: pack[p, PK_CB+1024l+128e+64j+b] =
        # C[rs][b,e] * cfac[l]
        for l in range(3):
            blk = np.broadcast_to(
                (C[rs].T * cfac[l])[:, None, :], (E, 2, ROWS)
            ).reshape(1, E * 2 * ROWS)
            off = PK_CB + l * E * 2 * ROWS
            pack[:, off : off + E * 2 * ROWS] = blk
        pack[0:ROWS, PK_ID : PK_ID + ROWS] = np.eye(ROWS, dtype=np.float32)
        pack[0:E, PK_CT : PK_CT + ROWS] = C[rs].T
        in_maps.append(
            {
                "pack": pack.astype(mm_np),
                "biasd": Bb,
                "w": W,
                "wpair": wpair,
            }
        )
    return in_maps


def run(inputs, mode=MODE, trace=False):
    """Returns (output [512,512] fp32, BassKernelResults)."""
    from concourse.bass_utils import run_bass_kernel_spmd

    nc = _get_nc(mode)
    in_maps = _prep_in_maps(inputs, mode)
    res = run_bass_kernel_spmd(nc, in_maps, list(range(NCORES)), trace=trace)
    out = np.concatenate([r["out"] for r in res.results], axis=0)
    return out, res


def kernel(**inputs) -> np.ndarray:
    out, _ = run(inputs)
    return out



# revision 48
# speedup vs baseline: 1.1567x; 1.0207x over previous
"""Blended-expert MLP (MoE routing) Trainium2 Bass kernel.

Math: reference computes, per layer,
    h = elu( einsum("bi,bio->bo", x, einsum("be,eio->bio", c, w)) + c @ b )
which factorizes as
    h = elu( sum_e (c[:,e] * x) @ W_e  +  c @ b )
(row-scaling commutes with the matmul), so per layer we scale X^T by
c_e on the vector engine (8 ops) and run 8 [rows,512]x[512,512]
matmuls plus one tiny K=8 matmul for the blended bias, ALL accumulating
into a single PSUM tile. Then ELU, then a PE transpose to produce the
next layer's stationary operand.

Sharding: data-parallel over the batch. B=512 rows split across 8
NeuronCores (64 rows each); the expert weights are replicated to every
core (fp16: 12 MB/core, fully SBUF-resident). No collectives (on-chip
AllReduce has a ~20us latency floor, worse than replication).

Layout per core:
  stationary operand = (c_e * X)^T chunks [128(i), 64(b)]
  moving operand     = W chunks  [128(i), 4096(e,o)] sliced per expert
  psum out           = [128(2 expert-halves x 64b), 512(o)], fp32

fp16 operands (not bf16): same DMA bytes and same 1-cycle/row matmul
rate, but 10 mantissa bits give ~7e-4 relative error vs ~6e-3.

Performance structure (measured ~50-54us on hardware, best 49.6us):
  ~7.5us fixed NEFF preamble | ~33.5us weight-DMA window (the wall:
  12 MB at ~360 GB/s/core, fully overlapped with compute) | ~4us
  compute tail | ~6us fixed all-engine end barrier.
Tricks that matter: PE warmup matmuls (HAM clock gate: cold PE runs at
1.2 GHz, warm 2.4 GHz), k-outer matmul order (each weight chunk's
matmuls fire on DMA arrival), even/odd expert pairs running
concurrently in the two column halves of the PE array (M=64 would
otherwise idle half the array), and a column-halved pipeline for the
psum-merge + ELU boundary chain.
"""

import numpy as np

B, E, D = 512, 8, 512
NCORES = 8
ROWS = B // NCORES  # 64
KC = D // 128  # 4 contraction chunks of 128

# pack tensor column layout (per 128 partitions)
PK_XT = 0  # [128, 256]: layer-1 x^T chunk k at cols [64k, 64k+64)
# c broadcast, one block per layer; col 1024l+128e+64j+b = C[b,e] times the
# layer's weight-dequant factor (s_w*1024 for int8 layers 0-1, 1 for fp16
# layer 2) - folding the scale here keeps the PSUM/evacuation path identical
PK_CB = 256
PK_ID = PK_CB + 3 * E * 2 * ROWS  # [64, 64]: identity, partitions 0..63
PK_CT = PK_ID + ROWS  # [8, 64]: coef^T, partitions 0..7
PCK = PK_CT + ROWS

MODE = "f16"
N8_LAYERS = 2  # layers 0..N8_LAYERS-1 ship weights as int8 byte-pairs
# pair chunks whose lo-byte extract runs on DVE (rest ACT): ACT's 1.9us
# per op can't keep the 1.45us/chunk DMA cadence, so DVE takes most
LO_DVE = {0, 1, 2, 4, 5}

_NC_CACHE = {}


def _mmdt(mybir, mode):
    return {
        "f32": mybir.dt.float32,
        "f32r": mybir.dt.float32r,
        "bf16": mybir.dt.bfloat16,
        "f16": mybir.dt.float16,
    }[mode]


def _build(mode):
    from contextlib import ExitStack

    import concourse.bacc as bacc
    import concourse.mybir as mybir
    import concourse.tile as tile

    f32 = mybir.dt.float32
    mmdt = _mmdt(mybir, mode)
    Alu = mybir.AluOpType
    Act = mybir.ActivationFunctionType

    # Bacc (not raw Bass): its compile() legalizes the TRN2 one-sync-wait-
    # per-instruction limit by splitting excess waits into EventSemaphores
    nc = bacc.Bacc()
    u16 = mybir.dt.uint16
    u8 = mybir.dt.uint8
    N8C = N8_LAYERS * KC  # int8 pair chunks (layers 0-1)
    pack_d = nc.declare_dram_parameter("pack", [128, PCK], mmdt, isOutput=False)
    bias_d = nc.declare_dram_parameter("biasd", [E, 3 * D], mmdt, isOutput=False)
    # layers 0-1: uint16 byte pairs (lo byte = experts 0-3, hi = 4-7),
    # widened on-chip to w_q/1024 in fp16; layer 2: direct fp16
    wpair_d = nc.declare_dram_parameter(
        "wpair", [N8C, 128, E * D // 2], u16, isOutput=False
    )
    w_d = nc.declare_dram_parameter(
        "w", [3 - N8_LAYERS, D, E * D], mmdt, isOutput=False
    )
    out_d = nc.declare_dram_parameter("out", [ROWS, D], f32, isOutput=True)
    HED = E * D // 2

    with ExitStack() as ctx:
        tc = ctx.enter_context(tile.TileContext(nc))
        const = ctx.enter_context(tc.tile_pool(name="const", bufs=1))
        wpool = ctx.enter_context(tc.tile_pool(name="wp", bufs=4))
        prpool = ctx.enter_context(tc.tile_pool(name="prp", bufs=8))
        wfpool = ctx.enter_context(tc.tile_pool(name="wfp", bufs=6))
        spool = ctx.enter_context(tc.tile_pool(name="sp", bufs=24))
        hpool = ctx.enter_context(tc.tile_pool(name="hp", bufs=2))
        xpool = ctx.enter_context(tc.tile_pool(name="xp", bufs=2))
        acc_ps = ctx.enter_context(tc.tile_pool(name="acc", bufs=3, space="PSUM"))
        pt_ps = ctx.enter_context(tc.tile_pool(name="pt", bufs=3, space="PSUM"))
        wm_ps = ctx.enter_context(tc.tile_pool(name="wm", bufs=1, space="PSUM"))

        # PE warmup: garbage matmuls on a zeroed tile (output never read),
        # emitted first so the HAM clock gate reaches 2.4 GHz before the
        # first real matmul (cold PE at 1.2 GHz otherwise doubles every
        # matmul). gpsimd is free right after its ~3.3us start preamble, so
        # it provides the earliest possible writer for the warm tile.
        warm = const.tile([128, ROWS + D], mmdt)
        nc.gpsimd.memset(warm[:], 0.0)
        wps = wm_ps.tile([ROWS, D], f32, tag="warm")
        for _ in range(14):
            nc.tensor.matmul(
                wps[:], warm[:, 0:ROWS], warm[:, ROWS:], start=True, stop=True
            )

        pack_t = const.tile([128, PCK], mmdt)
        pack_dma = nc.sync.dma_start(pack_t[:], pack_d[:])
        bias_t = const.tile([E, 3 * D], mmdt)
        nc.gpsimd.dma_start(bias_t[:], bias_d[:])

        coeft_ap = pack_t[0:E, PK_CT : PK_CT + ROWS]
        ident_ap = pack_t[0:ROWS, PK_ID : PK_ID + ROWS]
        xt_tile, xt_off = pack_t, PK_XT  # current x^T source: [128, 256] at offset

        # all weight-chunk DMAs up-front; the HWDGE lane round-robin plus
        # issue order paces them in consumption order at full bandwidth
        # (explicit chaining adds ~2us completion-latency per hop - worse)
        raw_pairs = []
        all_wts = []
        for layer in range(3):
            for k in range(KC):
                c = layer * KC + k
                rs_ = slice(128 * k, 128 * (k + 1))
                if layer < N8_LAYERS:
                    pr = prpool.tile([128, HED], u16, tag="wp")
                    nc.sync.dma_start(pr[:], wpair_d[c, :, :])
                    raw_pairs.append(pr)
                    all_wts.append(None)  # widened per layer below
                else:
                    wt = wpool.tile([128, E * D], mmdt, tag="w")
                    l2 = layer - N8_LAYERS
                    if layer == 2 and k == KC - 1:
                        # split the final chunk: its first half (experts
                        # 0-3) lands earlier, so only experts 4-7's last
                        # matmuls gate on the very last transfer
                        nc.sync.dma_start(
                            wt[:, 0:HED], w_d[l2, rs_, 0:HED]
                        )
                        nc.sync.dma_start(wt[:, HED:], w_d[l2, rs_, HED:])
                    else:
                        nc.sync.dma_start(wt[:], w_d[l2, rs_, :])
                    all_wts.append(wt)

        def widen(c):
            """Widen pair chunk c to fp16 w_q/1024 (exact in fp16)."""
            pr = raw_pairs[c]
            wf = wfpool.tile([128, E * D], mmdt, tag="wf")
            # hi: v/(256*1024) - 0.125 = (w_q + lo/256)/1024, lo-leakage
            # pre-compensated on the host
            nc.vector.tensor_scalar(
                wf[:, HED:], pr[:], 1.0 / (256.0 * 1024.0), 0.125,
                Alu.mult, Alu.subtract,
            )
            # lo: u8/1024 - 0.125 from the stride-2 byte view
            lov = pr[:].bitcast(u8).rearrange("p (n two) -> p two n", two=2)
            if c in LO_DVE:
                nc.vector.tensor_scalar(
                    wf[:, 0:HED], lov[:, 0, :], 1.0 / 1024.0, 0.125,
                    Alu.mult, Alu.subtract,
                )
            else:
                nc.scalar.activation(
                    wf[:, 0:HED], lov[:, 0, :], Act.Copy,
                    scale=1.0 / 1024.0, bias=-0.125,
                )
            return wf

        for layer in range(3):
            if layer < N8_LAYERS:
                wts = [widen(layer * KC + k) for k in range(KC)]
            else:
                wts = all_wts[layer * KC : (layer + 1) * KC]

            # scale x^T by c_e along the batch (free) dim: one DVE op per
            # expert over all 4 chunks at once
            # per-chunk rescale: TT(e,k) gates only on evacuation k of the
            # previous layer's transpose, and matmul (e,k) gates only on
            # TT(e,k) (subtile column tracking), so the boundary pipelines
            # at chunk granularity. c-broadcast is stored once (64 cols/e).
            # per chunk-pair rescale: TT(e,half) gates on the first/last two
            # transpose evacuations only, and matmul (e,k) gates on its half
            # (subtile column tracking)
            scaled = []
            for e in range(E):
                sc = spool.tile([128, KC * ROWS], mmdt, tag="sc")
                for half in range(2):
                    lo, hi = 2 * ROWS * half, 2 * ROWS * (half + 1)
                    cbo = PK_CB + 2 * ROWS * (E * layer + e)
                    nc.vector.tensor_tensor(
                        out=sc[:, lo:hi],
                        in0=xt_tile[:, xt_off + lo : xt_off + hi],
                        in1=pack_t[:, cbo : cbo + 2 * ROWS],
                        op=Alu.mult,
                    )
                scaled.append(sc)

            # one accumulation group: 32 expert matmuls + bias matmul (K=8).
            # k-outer order: each weight chunk's 8 expert matmuls fire as
            # soon as that chunk's DMA lands, overlapping the next transfer.
            # Even/odd experts run CONCURRENTLY in the two column halves of
            # the PE array (tile_position), since M=64 only fills half the
            # array; the partition halves of acc are summed afterwards.
            acc = acc_ps.tile([2 * ROWS, D], f32, tag="acc")
            # bias matmul opens the even-half group so the even half is done
            # (and can evacuate) while the last odd matmuls still run
            nc.tensor.matmul(
                acc[0:ROWS, :],
                coeft_ap,
                bias_t[:, D * layer : D * (layer + 1)],
                start=True,
                stop=False,
                tile_position=(0, 0),
                skip_group_check=True,
            )
            for k in range(KC):
                for e in range(E):
                    half = e % 2
                    nc.tensor.matmul(
                        acc[half * ROWS : (half + 1) * ROWS, :],
                        scaled[e][:, ROWS * k : ROWS * (k + 1)],
                        wts[k][:, D * e : D * (e + 1)],
                        start=(k == 0 and e == 1),
                        stop=(k == KC - 1 and e >= E - 2),
                        tile_position=(0, half * ROWS),
                        skip_group_check=True,
                    )
            # evacuate even half (ACT) + merge halves (DVE) + elu + transpose,
            # pipelined per 128-column quarter: transpose k consumes exactly
            # quarter k, so each quarter flows through the whole boundary
            # chain independently
            t0 = hpool.tile([ROWS, D], f32, tag="t0")
            hpre = hpool.tile([ROWS, D], f32, tag="hpre")
            HD = D // 2
            if layer < 2:
                # keep the PE clock warm across the elu/transpose boundary
                # (a >3.4us PE-idle window would re-throttle to 1.2 GHz)
                for _ in range(8):
                    nc.tensor.matmul(
                        wps[:], warm[:, 0:ROWS], warm[:, ROWS:],
                        start=True, stop=True,
                    )

            if layer < 2:
                # per quarter q: copy+merge, elu(x)=max(x,0)+min(exp(x)-1,0),
                # then transpose + evacuation - all stages pipeline across
                # quarters on alternating engines
                ex = hpool.tile([ROWS, D], f32, tag="ex")
                h = hpool.tile([ROWS, D], mmdt, tag="h")
                xt_t = xpool.tile([128, KC * ROWS], mmdt, tag="xt")
                for q in range(KC):
                    qs = slice(128 * q, 128 * (q + 1))
                    nc.scalar.copy(t0[:, qs], acc[0:ROWS, qs])
                    nc.vector.tensor_tensor(
                        out=hpre[:, qs], in0=t0[:, qs], in1=acc[ROWS:, qs],
                        op=Alu.add,
                    )
                    nc.scalar.activation(ex[:, qs], hpre[:, qs], Act.Exp)
                    nc.vector.tensor_scalar(
                        ex[:, qs], ex[:, qs], 1.0, 0.0, Alu.subtract, Alu.min
                    )
                    nc.vector.scalar_tensor_tensor(
                        out=h[:, qs],
                        in0=hpre[:, qs],
                        scalar=0.0,
                        in1=ex[:, qs],
                        op0=Alu.max,
                        op1=Alu.add,
                    )
                    pt = pt_ps.tile([128, ROWS], mmdt, tag="pt")
                    nc.tensor.transpose(pt[:], h[:, qs], ident_ap)
                    dst = xt_t[:, ROWS * q : ROWS * (q + 1)]
                    if q % 2 == 0:
                        nc.scalar.copy(dst, pt[:])
                    else:
                        nc.vector.tensor_copy(dst, pt[:])
                xt_tile, xt_off = xt_t, 0
            else:
                # stream the output per column half, right behind the merge
                for c in range(2):
                    cs = slice(HD * c, HD * (c + 1))
                    nc.scalar.copy(t0[:, cs], acc[0:ROWS, cs])
                    nc.vector.tensor_tensor(
                        out=hpre[:, cs], in0=t0[:, cs], in1=acc[ROWS:, cs],
                        op=Alu.add,
                    )
                    nc.sync.dma_start(out_d[:, cs], hpre[:, cs])

    nc.compile()
    return nc


def _get_nc(mode):
    if mode not in _NC_CACHE:
        _NC_CACHE[mode] = _build(mode)
    return _NC_CACHE[mode]


def _prep_in_maps(inputs, mode):
    X = np.asarray(inputs["X"], np.float32)
    C = np.asarray(inputs["blending_coef"], np.float32)
    ws = [np.asarray(inputs[f"w_l{i}"], np.float32) for i in (1, 2, 3)]
    bs = [np.asarray(inputs[f"b_l{i}"], np.float32) for i in (1, 2, 3)]
    mm_np = np.float16

    # layers 0..N8_LAYERS-1: int8 byte-pairs; the rest: direct fp16.
    # W[l][i, e*D+o] = w_l[e, i, o]
    HED = E * D // 2
    sw = np.ones(3, np.float32)
    wpair = np.zeros((N8_LAYERS * KC, 128, HED), np.uint16)
    for l in range(N8_LAYERS):
        Wl = ws[l].transpose(1, 0, 2).reshape(D, E * D)
        sw[l] = max(np.abs(Wl).max() / 127.0, 1e-30)
        Wq = (Wl / sw[l]).astype(np.float64)
        for k in range(KC):
            sub = Wq[128 * k : 128 * (k + 1)]
            # lo byte = experts 0-3; hi byte = experts 4-7. On-chip hi
            # extract is v/256 - 128 (in w_q units), leaking lo/256: pre-
            # subtract it before rounding.
            a = (np.round(sub[:, :HED]).clip(-127, 127) + 128.0).astype(
                np.uint16
            )
            b = np.round(sub[:, HED:] + 128.0 - a / 256.0).clip(0, 255)
            wpair[l * KC + k] = a | (b.astype(np.uint16) << 8)
    W = np.stack(
        [
            w.transpose(1, 0, 2).reshape(D, E * D)
            for w in ws[N8_LAYERS:]
        ]
    ).astype(mm_np)
    Bb = np.concatenate(bs, axis=1).astype(mm_np)  # [E, 3*D]

    # per-layer c-broadcast factor: widened weights are w_q/1024, so the
    # xs side carries s_w*1024 (safely inside fp16 normal range)
    cfac = np.where(np.arange(3) < N8_LAYERS, sw * 1024.0, 1.0)

    in_maps = []
    for c in range(NCORES):
        rs = slice(c * ROWS, (c + 1) * ROWS)
        pack = np.zeros((128, PCK), np.float32)
        # xt chunks: pack[p, 64k+b] = X[rows][b, 128k+p]
        xt = np.ascontiguousarray(X[rs].T)  # [512, 64]
        pack[:, PK_XT : PK_XT + KC * ROWS] = (
            xt.reshape(KC, 128, ROWS).transpose(1, 0, 2).reshape(128, KC * ROWS)
        )
        # c broadcast per layer: pack[p, PK_CB+1024l+128e+64j+b] =
        # C[rs][b,e] * cfac[l]
        for l in range(3):
            blk = np.broadcast_to(
                (C[rs].T * cfac[l])[:, None, :], (E, 2, ROWS)
            ).reshape(1, E * 2 * ROWS)
            off = PK_CB + l * E * 2 * ROWS
            pack[:, off : off + E * 2 * ROWS] = blk
        pack[0:ROWS, PK_ID : PK_ID + ROWS] = np.eye(ROWS, dtype=np.float32)
        pack[0:E, PK_CT : PK_CT + ROWS] = C[rs].T
        in_maps.append(
            {
                "pack": pack.astype(mm_np),
                "biasd": Bb,
                "w": W,
                "wpair": wpair,
            }
        )
    return in_maps


def run(inputs, mode=MODE, trace=False):
    """Returns (output [512,512] fp32, BassKernelResults)."""
    from concourse.bass_utils import run_bass_kernel_spmd

    nc = _get_nc(mode)
    in_maps = _prep_in_maps(inputs, mode)
    res = run_bass_kernel_spmd(nc, in_maps, list(range(NCORES)), trace=trace)
    out = np.concatenate([r["out"] for r in res.results], axis=0)
    return out, res


def kernel(**inputs) -> np.ndarray:
    out, _ = run(inputs)
    return out

